# revision 48
# baseline (speedup 1.0000x reference)
"""Fused multi-head attention for Trainium2, SPMD over 8 NeuronCores — v3.

Sharding: core c handles batch c//2, query rows [(c%2)*1024, ...) — data
parallel over batch x query-length; K/V projections recomputed by the two
cores sharing a batch.

v3 changes vs v2 (238us traced -> ~217us):
- ALL weights host-prepped to their SBUF layouts; inputs chunk-major
  ([n, 128, 4, cols]) so every load is a few wide contiguous DMA
  descriptors.  Weights on the SWDGE (gpsimd) ring, x-inputs on the
  sync HWDGE ring, both in consumption order (a second HWDGE ring just
  steals bandwidth from the critical path).
- PE pre-warm: cheap 1-col dummy matmuls during the initial DMA wait so
  HAM releases the clock gate before the first projection.
- V-projection front-loaded into pair-0's exp-gated ramp, two chunks
  per iteration, paced to the xv quarter-chunk arrivals.
- Softmax exp split ACT/DVE: pairs 1-3 run every 3rd tile on DVE via
  Schraudolph bf16 bitcast (ACT alone = 1 elem/cycle/lane = 146us,
  comparable to the whole PE stream).
- oT2 packed two-heads-per-128-partitions (head-odd placed by a
  partition-shift SBUF->SBUF DMA after normalize) -> out-projection
  contracts K=128 densely: 4 matmuls per l-chunk instead of 8.
- Out-projection emitted pr-major with its 8 PSUM accumulator halves
  reusing the scores ring + av tags (per-tile WAR, no pool-close
  barrier), so 24 of 32 matmuls overlap pair 3's normalize chain.
- Output partition-major + batched 2-half store (8 per-chunk stores
  paid ~0.7us of descriptor processing each).

PSUM budget (8 banks): sc ring 2 x [128,2,512] f32 (4) + av_e/av_o
[128,1024] f32 (4).
"""

import numpy as np

B, L, S, D, H, E = 4, 2048, 2048, 512, 8, 64
LC = L // 2
N_CORES = 8
SC = S // 128        # 16 s-chunks
QC = LC // 512       # 2 q-chunks of 512

# A@V emission lag in qc-iterations.  LAG0 must keep each pair-0 A@V pop
# BEHIND its proj_v in the (in-order) PE stream: chunk j is projected at
# the end of iter j+4, its first A@V pops at iter (2j+LAG0)//2.
LAG0 = 10
LAGN = 4

# Schraudolph constants for bf16-via-int16: I = round(x * SCHRA_A + SCHRA_B),
# bitcast int16 -> bf16 gives ~exp(x/8) (scale 1/sqrt(E) folded into A).
SCHRA_A = 128.0 / (8.0 * np.log(2.0))
SCHRA_B = 127.0 * 128.0 - 7.5  # DVE f32->int cast rounds to nearest

_cached = None


def _dve_exp(p, i, qc):
    # (pair, chunk, qc) combos whose exp runs on DVE (Schraudolph) instead
    # of ACT: every 3rd tile in pairs 1-3 (pair 0's ramp keeps DVE free for
    # bias adds; ACT is the softmax-exp bottleneck otherwise).
    return p >= 1 and (2 * i + qc) % 3 == 2


def _build_bass():
    import concourse.bacc as bacc
    import concourse.mybir as mybir
    from concourse.tile import TileContext

    f32 = mybir.dt.float32
    bf16 = mybir.dt.bfloat16
    i16 = mybir.dt.int16
    AF = mybir.ActivationFunctionType
    ALU = mybir.AluOpType

    nc = bacc.Bacc("TRN2", target_bir_lowering=False, debug=False,
                   num_devices=N_CORES)

    # Inputs pre-TRANSPOSED on the host to [128, 4, rows] (d-chunk on
    # partitions); weights pre-arranged to their SBUF layouts.  Everything
    # loads as a plain wide DMA.
    # Inputs are chunk-MAJOR ([n_chunks, 128, 4, cols]) so each chunked
    # load is one contiguous block per partition (few big DMA
    # descriptors instead of hundreds of 2KB ones).
    xq = nc.dram_tensor("xq", [2, 128, 4, 512], bf16, kind="ExternalInput")
    xk = nc.dram_tensor("xk", [4, 128, 4, 512], bf16, kind="ExternalInput")
    xv = nc.dram_tensor("xv", [4, 128, 4, 512], bf16, kind="ExternalInput")
    wq = nc.dram_tensor("wq", [128, 4, D], bf16, kind="ExternalInput")
    wk = nc.dram_tensor("wk", [128, 4, D], bf16, kind="ExternalInput")
    wv = nc.dram_tensor("wv", [128, 4, 520], bf16, kind="ExternalInput")
    wo = nc.dram_tensor("wo", [128, 4, D], bf16, kind="ExternalInput")
    bq = nc.dram_tensor("bq", [128, 4], f32, kind="ExternalInput")
    bk = nc.dram_tensor("bk", [128, 4], f32, kind="ExternalInput")
    bv = nc.dram_tensor("bv", [1, 8 * 65], f32, kind="ExternalInput")
    bo = nc.dram_tensor("bo", [1, D], f32, kind="ExternalInput")
    # Output is partition-major [128, LC/128, D] (one contiguous 16KB
    # block per partition for the single batched store); the host
    # reassembles row order.
    y = nc.dram_tensor("y", [128, LC // 128, D], f32, kind="ExternalOutput")

    import contextlib
    with TileContext(nc) as tc, contextlib.ExitStack() as ctx:
        persist = ctx.enter_context(tc.tile_pool(name="persist", bufs=1))

        wk_sb = persist.tile([128, 4, 512], bf16)
        wq_sb = persist.tile([128, 4, 512], bf16)
        wv_sb = persist.tile([128, 4, 520], bf16)
        wo_sb = persist.tile([128, 4, 512], bf16)
        bqT = persist.tile([128, 4], f32)
        bkT = persist.tile([128, 4], f32)
        bv_bc = persist.tile([128, 520], f32)
        bo_bc = persist.tile([128, 512], f32)
        warm = persist.tile([128, 128], bf16)

        # Input tiles, chunked so compute can start on the first pieces
        # (separate tiles per chunk so consumers gate on exactly one DMA).
        xt = ctx.enter_context(tc.tile_pool(name="xt", bufs=1))
        xkTq = [xt.tile([128, 4, 512], bf16, name=f"xkT{q}")
                for q in range(4)]
        xqT0 = xt.tile([128, 4, 512], bf16)
        xqT1 = xt.tile([128, 4, 512], bf16)
        xvTq = [xt.tile([128, 4, 512], bf16, name=f"xvT{q}")
                for q in range(4)]

        # Weights ride the SWDGE (gpsimd) ring, x-inputs the HWDGE (sync)
        # ring, both in consumption order — the two rings transfer
        # concurrently, and HWDGE rings are serial within themselves so
        # ordering actually prioritizes.  (Two concurrent HWDGE rings
        # corrupt on HW, so everything HWDGE stays on sync.)
        # memset FIRST: the PE warm-up dummies depend on it.
        nc.gpsimd.memset(warm, 0.0)
        nc.gpsimd.dma_start(out=wk_sb, in_=wk[:, :, :])
        nc.gpsimd.dma_start(out=bkT, in_=bk[:, :])
        nc.gpsimd.dma_start(out=wq_sb, in_=wq[:, :, :])
        nc.gpsimd.dma_start(out=bqT, in_=bq[:, :])
        nc.gpsimd.dma_start(out=wv_sb, in_=wv[:, :, :])
        nc.gpsimd.dma_start(out=bv_bc, in_=bv[0:1, :].broadcast_to((128, 520)))
        nc.gpsimd.dma_start(out=wo_sb, in_=wo[:, :, :])
        nc.gpsimd.dma_start(out=bo_bc, in_=bo[0:1, :].broadcast_to((128, 512)))
        nc.sync.dma_start(out=xkTq[0], in_=xk[0])
        nc.sync.dma_start(out=xkTq[1], in_=xk[1])
        nc.sync.dma_start(out=xqT0, in_=xq[0])
        nc.sync.dma_start(out=xqT1, in_=xq[1])
        nc.sync.dma_start(out=xkTq[2], in_=xk[2])
        nc.sync.dma_start(out=xkTq[3], in_=xk[3])
        nc.sync.dma_start(out=xvTq[0], in_=xv[0])
        nc.sync.dma_start(out=xvTq[1], in_=xv[1])
        nc.sync.dma_start(out=xvTq[2], in_=xv[2])
        nc.sync.dma_start(out=xvTq[3], in_=xv[3])

        # Long-lived attention operands (packed head-pair layout, no pads).
        attn = ctx.enter_context(tc.tile_pool(name="attn", bufs=1))
        qT = attn.tile([128, 4, LC], bf16)     # pair m: head 2m rows 0:64, 2m+1 rows 64:128
        kT = attn.tile([128, 4, S], bf16)
        vaug = attn.tile([128, SC, 8 * 65], bf16)  # per s-chunk: 8x [V_h | 1]
        # Normalized attention out, PACKED per pair: rows 0:64 = head 2p's
        # E dims, rows 64:128 = head 2p+1's (written by a partition-shift
        # SBUF->SBUF DMA).  The out-projection contracts K=128 densely.
        oT2 = attn.tile([128, 4, LC], bf16)
        yfull = attn.tile([128, LC // 128, 512], f32)

        with tc.tile_pool(name="scp", bufs=2, space="PSUM") as scp, \
             tc.tile_pool(name="avp", bufs=1, space="PSUM") as avp, \
             tc.tile_pool(name="pp", bufs=LAG0 + 4) as pp, \
             tc.tile_pool(name="zrp", bufs=2) as zrp:

            # PE pre-warm: cheap dummy matmuls (1-col stationary so the
            # implicit LDWEIGHTS is ~free; one PSUM tile, same-engine WAW
            # only) keep the PE busy through the initial DMA wait so HAM
            # releases the clock gate before the first projection.
            pswarm = scp.tile([128, 1024], f32, tag="sc", name="wm")
            for w in range(24):
                nc.tensor.matmul(pswarm[0:1, 0:64], warm[:, 0:1],
                                 warm[:, 0:64], start=True, stop=True)

            def proj_k_n(m, n):
                # kT columns [n*1024, (n+1)*1024) for head-pair m.
                # half-OUTER: the first 4 matmuls gate on one 0.5MB xk
                # quarter, so the projection starts as soon as it lands.
                ps = scp.tile([128, 1024], f32, tag="sc", name=f"psk_{m}_{n}")
                for half in range(2):
                    xkc = xkTq[2 * n + half]
                    for k in range(4):
                        nc.tensor.matmul(
                            ps[:, half * 512:(half + 1) * 512],
                            wk_sb[:, k, m * 128:(m + 1) * 128],
                            xkc[:, k, :],
                            start=(k == 0), stop=(k == 3))
                nc.vector.tensor_add(
                    kT[:, m, 2 * n * 512:(2 * n + 2) * 512],
                    ps,
                    bkT[:, m:m + 1].to_broadcast((128, 1024)))

            def proj_q_h(m, half):
                xqc = xqT0 if half == 0 else xqT1
                ps = scp.tile([128, 1024], f32, tag="sc", name=f"psq_{m}_{half}")
                for k in range(4):
                    nc.tensor.matmul(
                        ps[:, 0:512],
                        wq_sb[:, k, m * 128:(m + 1) * 128],
                        xqc[:, k, :],
                        start=(k == 0), stop=(k == 3))
                nc.vector.tensor_add(
                    qT[:, m, half * 512:(half + 1) * 512],
                    ps[:, 0:512],
                    bqT[:, m:m + 1].to_broadcast((128, 512)))

            def proj_v(i):
                xvc = xvTq[i // 4]
                off = (i % 4) * 128
                ps = scp.tile([128, 1024], f32, tag="sc", name=f"psv_{i}")
                for k in range(4):
                    for half in range(2):
                        nc.tensor.matmul(
                            ps[:, half * 512:half * 512 + 260],
                            xvc[:, k, off:off + 128],
                            wv_sb[:, k, half * 260:(half + 1) * 260],
                            start=(k == 0), stop=(k == 3))
                nc.vector.tensor_add(
                    vaug[:, i, :].rearrange("p (a b) -> p a b", a=2),
                    ps.rearrange("p (a b) -> p a b", a=2)[:, :, 0:260],
                    bv_bc[:, :].rearrange("p (a b) -> p a b", a=2))

            # Work injected block-by-block at the END of chosen iterations of
            # pair p so projection matmuls never stall the exp stream.
            # Pair 0: q(0,1) asap (needed by iter 1's qc=1 scores), then one
            # V-proj chunk per iteration from iter 4 (xv/wv land ~11us), then
            # pair-1 K/Q.  Pairs 1-2: next pair's K/Q only.
            def pre_injections(p, i, qc):
                # Emitted BETWEEN the qc=0 and qc=1 iterations of pair 0's
                # iter 0: qT half 1 must be written before the qc=1 scores
                # read it (a later write is not a producer in program
                # order — the read would see uninitialized SBUF).
                if p == 0 and i == 0 and qc == 1:
                    proj_q_h(0, 1)

            def injections(p, i):
                # NOTE: i is the s-chunk index, 0..SC-1 (NOT the qc-step t).
                if p == 0:
                    # Iters 0-3 are exp-gated (V/injection data still in
                    # flight, PE ~70% idle): fill them with pair 0's second
                    # kT block and pair 1's Q-projections, unloading the
                    # pop-heavy iters 12-15.
                    if i == 1:
                        proj_k_n(0, 1)
                    # V-proj: two chunks per iter from iter 4, paced to the
                    # xv quarter-chunk DMA arrivals.  Chunk j lands at iter
                    # 4+j//2, safely before its first A@V pop (iter j+5).
                    if 4 <= i <= 11:
                        proj_v(2 * (i - 4))
                        proj_v(2 * (i - 4) + 1)
                    sched = {12: 0, 13: 1}
                    qsched = {2: 0, 3: 1}
                elif p == 3:
                    return
                else:
                    sched = {6: 0, 8: 1}
                    qsched = {11: 0, 12: 1}
                if i in sched:
                    proj_k_n(p + 1, sched[i])
                if i in qsched:
                    proj_q_h(p + 1, qsched[i])

            # Minimal prologue: scores iter (0, qc=0) only needs kT block 0
            # and qT half 0; the rest is injected.
            proj_k_n(0, 0)
            proj_q_h(0, 0)

            # Global rolling A@V deferral with CROSS-PAIR carry: each A@V
            # pair is emitted LAG qc-iters after its exp, and a pair's
            # leftover backlog drains during the next pair's score/exp
            # stream.  The softmax-normalize for a pair is emitted right
            # after its last A@V drains.
            pending = []

            def emit_av(p, i, qc, pt, av_e, av_o):
                he, ho = 2 * p, 2 * p + 1
                nc.tensor.matmul(
                    av_e[0:65, qc * 512:(qc + 1) * 512],
                    vaug[:, i, he * 65:(he + 1) * 65],
                    pt[:, 0:512],
                    start=(i == 0), stop=(i == SC - 1))
                nc.tensor.matmul(
                    av_o[0:65, qc * 512:(qc + 1) * 512],
                    vaug[:, i, ho * 65:(ho + 1) * 65],
                    pt[:, 512:1024],
                    start=(i == 0), stop=(i == SC - 1))
                if i == SC - 1 and qc == QC - 1:
                    normalize(p, av_e, av_o)

            def normalize(p, av_e, av_o):
                # z sits at av row 0 (augmented V column FIRST — DVE ops
                # only work at partition base 0).  Normalize into a base-0
                # temp (row 0 = z/z junk), then a partition-shift
                # SBUF->SBUF DMA packs rows 1:65 into oT2 at the head's
                # 64-row slot.  The e/o chains are interleaved so the DVE
                # and gpsimd stages pipeline instead of serializing.
                zis, bcs, ots = [], [], []
                for side, av in ((0, av_e), (1, av_o)):
                    h = 2 * p + side
                    zinv = zrp.tile([1, 1024], f32, tag="zinv", name=f"zi_{h}")
                    nc.vector.reciprocal_approx_fast(
                        out=zinv[0:1, :], in_=av[0:1, :])
                    zis.append(zinv)
                for side, av in ((0, av_e), (1, av_o)):
                    h = 2 * p + side
                    bcinv = zrp.tile([65, 1024], f32, tag="bcinv",
                                     name=f"bc_{h}")
                    nc.gpsimd.partition_broadcast(bcinv, zis[side][0:1, :])
                    bcs.append(bcinv)
                for side, av in ((0, av_e), (1, av_o)):
                    h = 2 * p + side
                    ot = zrp.tile([65, 1024], bf16, tag="ot", name=f"ot_{h}")
                    nc.vector.tensor_mul(ot, av[0:65, :], bcs[side])
                    nc.gpsimd.dma_start(
                        out=oT2[side * 64:side * 64 + 64, p, :],
                        in_=ot[1:65, :])

            for p in range(4):
                av_e = avp.tile([128, 1024], f32, tag="av_e",
                                name=f"av_{2 * p}")
                av_o = avp.tile([128, 1024], f32, tag="av_o",
                                name=f"av_{2 * p + 1}")
                for i in range(SC):
                    # Both qc-halves' score matmuls back-to-back: the
                    # second pair's LDWEIGHTS (row-disjoint from the
                    # in-flight matmuls) pulls ahead instead of waiting
                    # behind a full-row A@V.  Exps then pops follow.
                    scs = []
                    for qc in range(QC):
                        pre_injections(p, i, qc)
                        sc = scp.tile([128, 1024], f32, tag="sc",
                                      name=f"sc_{p}_{i}_{qc}")
                        nc.tensor.matmul(
                            sc[:, 0:512],
                            kT[0:64, p, i * 128:(i + 1) * 128],
                            qT[0:64, p, qc * 512:(qc + 1) * 512],
                            start=True, stop=True)
                        nc.tensor.matmul(
                            sc[:, 512:1024],
                            kT[64:128, p, i * 128:(i + 1) * 128],
                            qT[64:128, p, qc * 512:(qc + 1) * 512],
                            start=True, stop=True)
                        scs.append(sc)
                    for qc in range(QC):
                        pt = pp.tile([128, 1024], bf16, tag="p",
                                     name=f"p_{p}_{i}_{qc}")
                        if _dve_exp(p, i, qc):
                            nc.vector.tensor_scalar(
                                out=pt.bitcast(i16), in0=scs[qc],
                                scalar1=float(SCHRA_A), scalar2=float(SCHRA_B),
                                op0=ALU.mult, op1=ALU.add)
                        else:
                            nc.scalar.activation(out=pt, in_=scs[qc],
                                                 func=AF.Exp,
                                                 scale=float(1.0 / np.sqrt(E)))
                        pending.append((p, i, qc, pt, av_e, av_o))
                    t = 2 * i + 1
                    if p == 0:
                        # Ramp the lag down 1-per-iter once the V path is
                        # up, so the backlog drains smoothly.
                        lag = max(LAGN, LAG0 - max(0, t - 24))
                    elif p == 3:
                        # Ramp OUT at the very end so the last A@V (and
                        # the final normalize) land right after the last
                        # exp instead of trailing it by LAGN iters.
                        lag = max(0, LAGN - max(0, t - 27))
                    else:
                        lag = LAGN
                    while len(pending) > lag:
                        emit_av(*pending.pop(0))
                    injections(p, i)
            while pending:
                emit_av(*pending.pop(0))

            # ---- Output projection: Y = O @ Wo + bo.  K=128 per pair
            # (packed oT2) -> 4 matmuls per l-chunk.  Still INSIDE the pool
            # scope: the 8 PSUM accumulator halves reuse the scores ring
            # and the av tags, so bank recycling is tracked per-tile (a
            # pool close would barrier on ALL tiles, including pair 3's
            # normalize reads).  Emission is pr-major in bank-release
            # order; only the 8 pr=3 matmuls wait on the last normalize.
            ypt_a = scp.tile([128, 1024], f32, tag="sc", name="ypt_a")
            ypt_b = scp.tile([128, 1024], f32, tag="sc", name="ypt_b")
            ypav_e = avp.tile([128, 1024], f32, tag="av_e", name="ypav_e")
            ypav_o = avp.tile([128, 1024], f32, tag="av_o", name="ypav_o")
            yps = [ypt_a[:, 0:512], ypt_a[:, 512:1024],
                   ypt_b[:, 0:512], ypt_b[:, 512:1024],
                   ypav_e[:, 0:512], ypav_e[:, 512:1024],
                   ypav_o[:, 0:512], ypav_o[:, 512:1024]]
            for lcs in ((0, 1, 2, 3), (4, 5), (6, 7)):
                for pr in range(3):
                    for lc in lcs:
                        nc.tensor.matmul(
                            yps[lc], oT2[:, pr, lc * 128:(lc + 1) * 128],
                            wo_sb[:, pr, :],
                            start=(pr == 0), stop=False)
            for lc in range(LC // 128):
                nc.tensor.matmul(
                    yps[lc], oT2[:, 3, lc * 128:(lc + 1) * 128],
                    wo_sb[:, 3, :],
                    start=False, stop=True)
                nc.vector.tensor_add(yfull[:, lc, :], yps[lc], bo_bc)
                if lc == 3:
                    nc.sync.dma_start(out=y[:, 0:4, :], in_=yfull[:, 0:4, :])
            # Batched store in two halves; y is partition-major so each
            # is one contiguous 8KB block per partition, and the first
            # half launches while the pr=3 tail is still finishing.
            nc.sync.dma_start(out=y[:, 4:8, :], in_=yfull[:, 4:8, :])

    nc.compile()
    return nc


def _get_compiled():
    global _cached
    if _cached is None:
        _cached = _build_bass()
    return _cached


def make_in_maps(queries, keys, values, Wq, bq, Wk, bk, Wv, bv, Wo, bo):
    import ml_dtypes
    bf16 = ml_dtypes.bfloat16
    f = np.ascontiguousarray

    # Augment Wv/bv with a ones output column per head (FIRST, index 0):
    # the extra column of the A@V matmul accumulates the softmax
    # denominator z at av row 0, above the 64 value rows.
    wv_aug = np.zeros((D, 8 * 65), dtype=np.float32)
    bv_aug = np.zeros((1, 8 * 65), dtype=np.float32)
    wv_np = np.asarray(Wv, dtype=np.float32)
    bv_np = np.asarray(bv, dtype=np.float32).reshape(D)
    for h in range(8):
        wv_aug[:, h * 65 + 1:h * 65 + 65] = wv_np[:, h * 64:(h + 1) * 64]
        bv_aug[0, h * 65 + 1:h * 65 + 65] = bv_np[h * 64:(h + 1) * 64]
        bv_aug[0, h * 65] = 1.0
    queries = np.asarray(queries)

    def chunk4(x, nchunk, cols):
        # [rows, 512] -> transposed [128, 4, rows], then chunk-major
        # [nchunk, 128, 4, cols] contiguous bf16
        xb = np.asarray(x, dtype=np.float32).astype(bf16)
        t = xb.reshape(-1, 4, 128).transpose(2, 1, 0)
        return f(np.stack([t[:, :, i * cols:(i + 1) * cols]
                           for i in range(nchunk)]))

    def wprep(w, nchunk):
        # [nchunk*128, dout] -> [128, nchunk, dout] bf16 (SBUF layout)
        wb = np.asarray(w, dtype=np.float32).astype(bf16)
        return f(wb.reshape(nchunk, 128, -1).transpose(1, 0, 2))

    wk_p = wprep(Wk, 4)
    wq_p = wprep(Wq, 4)
    wv_p = wprep(wv_aug, 4)
    wo_p = wprep(np.asarray(Wo, dtype=np.float32), 4)
    bq_p = f(np.asarray(bq, dtype=np.float32).reshape(4, 128).T)
    bk_p = f(np.asarray(bk, dtype=np.float32).reshape(4, 128).T)
    bv_aug = f(bv_aug)

    in_maps = []
    for c in range(N_CORES):
        b, half = c // 2, c % 2
        in_maps.append({
            "xq": chunk4(queries[b, half * LC:(half + 1) * LC, :], 2, 512),
            "xk": chunk4(np.asarray(keys)[b], 4, 512),
            "xv": chunk4(np.asarray(values)[b], 4, 512),
            "wq": wq_p,
            "wk": wk_p,
            "wv": wv_p,
            "wo": wo_p,
            "bq": bq_p,
            "bk": bk_p,
            "bv": bv_aug,
            "bo": f(np.asarray(bo).reshape(1, D), dtype=np.float32),
        })
    return in_maps


def gather_out(results):
    out = np.empty((B, L, D), dtype=np.float32)
    for c in range(N_CORES):
        b, half = c // 2, c % 2
        yv = np.asarray(results[c]["y"])  # [128, LC/128, D] partition-major
        out[b, half * LC:(half + 1) * LC, :] = (
            yv.transpose(1, 0, 2).reshape(LC, D))
    return out


def kernel(queries, keys, values, Wq, bq, Wk, bk, Wv, bv, Wo, bo):
    from concourse.bass_utils import run_bass_kernel_spmd

    nc = _get_compiled()
    in_maps = make_in_maps(queries, keys, values, Wq, bq, Wk, bk, Wv, bv, Wo, bo)
    res = run_bass_kernel_spmd(nc, in_maps, core_ids=list(range(N_CORES)))
    return gather_out(res.results)
